# revision 5
# baseline (speedup 1.0000x reference)
"""Trainium2 Bass kernel for nn_BoundarySeg (segment_reduce).

out[b, j, 0:H]   = sum_{i>=j} A[b, j, i] * h[b, i, :]
out[b, j, H:2H]  = h[b, j, :] * sum_{i>=j} A[b, j, i]

Shapes: A [8, 2048, 2048] f32, h [8, 2048, 256] f32 -> out [8, 2048, 512] f32.
Sharding: data-parallel over batch; core c computes batch c.

Per-core algorithm (L=2048 in 16 tiles of 128, H=256):
  - h loads on the gpsimd SWDGE queue (its own DMA path) with an
    in-flight fp32->f32r cast, with a ones column at [.., 256] so the
    masked row-sum falls out of the main matmul as an extra column
    (N=258 keeps f32r at 1 cycle/row).
  - A panels jc=0..15: only the upper panel A[jc, jc:] is loaded (two
    HWDGE rings, byte-balanced, GROUP=8 tiles per chunk), transposed on
    PE in fp32 through PSUM and rounded to f32r during the PSUM->SBUF
    copy (the BIR verifier requires f32r matmul inputs to come from a
    rounding producer); the diagonal-containing chunk is masked (keep
    i >= j) by that copy via tensor_tensor on DVE.
  - acc[j, n] += At_block^T @ h over i-tiles >= jc (f32r, N=258).
    first half = acc[:, 0:256] (DVE copy to SBUF); second half =
    h[j, :] * acc[:, 256] (ACT activation with SBUF rowsum scale);
    single 256 KB store per panel on the gpsimd SWDGE queue.
  - matmuls run two panels behind transposes so the in-order PE queue
    never stalls on PSUM->SBUF copies.
"""

import os
import sys

import numpy as np

sys.path.insert(0, "/opt/trn_rl_repo")

import concourse.bass as bass  # noqa: E402
import concourse.bacc as bacc  # noqa: E402
import concourse.tile as tile  # noqa: E402
from concourse import mybir  # noqa: E402
from concourse.bass_utils import run_bass_kernel_spmd  # noqa: E402
from concourse.masks import make_identity, make_lower_triangular  # noqa: E402

B, L, H = 8, 2048, 256
P = 128
GROUP = 8  # 128-col transposes batched per PSUM tile / copy

DT = mybir.dt.float32
F32R = mybir.dt.float32r

# Results of the last run (exec_time_ns etc.) for the test harness.
LAST_RESULTS = None
_NC_CACHE = {}


def _build_nc(L=L, H=H):
    NT = L // P
    HE = H + 2  # even N for f32r; col H = ones (rowsum), col H+1 unused

    nc = bacc.Bacc(None, target_bir_lowering=False)
    a_dram = nc.dram_tensor("a", [L, L], DT, kind="ExternalInput")
    h_dram = nc.dram_tensor("h", [L, H], DT, kind="ExternalInput")
    out_dram = nc.dram_tensor("out", [L, 2 * H], DT, kind="ExternalOutput")

    with tile.TileContext(nc) as tc:
        with (
            tc.tile_pool(name="const", bufs=1) as const_pool,
            tc.tile_pool(name="hpool", bufs=1) as h_pool,
            tc.tile_pool(name="apanel", bufs=5) as a_pool,
            tc.tile_pool(name="atT", bufs=3) as at_pool,
            tc.tile_pool(name="tp", bufs=3, space=bass.MemorySpace.PSUM) as tp_pool,
            tc.tile_pool(name="acc", bufs=2, space=bass.MemorySpace.PSUM) as acc_pool,
            tc.tile_pool(name="outsb", bufs=3) as out_pool,
            tc.tile_pool(name="small", bufs=3) as small_pool,
        ):
            identity = const_pool.tile([P, P], DT)
            make_identity(nc, identity[:])
            # Mask for the *transposed* diagonal block ([i(part), j(free)],
            # keep i >= j -> lower triangular); columns P.. multiply by 1.0.
            # Bounced through DVE so consumers depend on DVE, not Pool.
            mask_src = const_pool.tile([P, P], DT)
            make_lower_triangular(nc, mask_src[:], val=1.0, diag=True)
            cmask = const_pool.tile([P, GROUP * P], DT)
            nc.vector.tensor_copy(cmask[:, 0:P], mask_src[:])
            nc.vector.memset(cmask[:, P : GROUP * P], 1.0)

            # h on the gpsimd SWDGE queue (own DMA path, starts early);
            # landed directly as f32r bits.  Ones columns set first (the
            # DMA only writes [.., 0:H]).
            h_all = h_pool.tile([P, NT, HE], F32R)
            ones_stage = const_pool.tile([P, NT * 2], DT)
            nc.vector.memset(ones_stage[:], 1.0)
            nc.vector.tensor_copy(
                h_all[:, :, H:HE], ones_stage[:].rearrange("p (t c) -> p t c", c=2)
            )
            h_re = h_dram[:].rearrange("(t p) n -> p t n", p=P)
            for t0, t1 in ((0, 2), (2, 4), (4, 8), (8, 12), (12, 16)):
                nc.gpsimd.dma_start(
                    out=h_all[:, t0:t1, 0:H], in_=h_re[:, t0:t1, :]
                )

            # Warmup transpose: absorbs the Pool->PE wait for `identity`.
            wtp = tp_pool.tile([P, GROUP * P], DT, tag="tp")
            nc.tensor.transpose(wtp[:, 0:P], identity[:], identity[:])

            def matmuls_and_store(jc, atT):
                ntiles = NT - jc
                acc = acc_pool.tile([P, HE], DT, tag="acc")
                for k in range(ntiles):
                    nc.tensor.matmul(
                        acc[:],
                        atT[:, k * P : (k + 1) * P],
                        h_all[:, jc + k, :],
                        start=(k == 0),
                        stop=(k == ntiles - 1),
                    )
                out_sb = out_pool.tile([P, 2 * H], DT, tag="outsb")
                rowsum = small_pool.tile([P, 1], DT, tag="rowsum")
                nc.scalar.copy(rowsum[:], acc[:, H : H + 1])
                nc.scalar.activation(
                    out_sb[:, H : 2 * H],
                    h_all[:, jc, 0:H].bitcast(DT),
                    mybir.ActivationFunctionType.Identity,
                    scale=rowsum[:],
                )
                nc.vector.tensor_copy(out_sb[:, 0:H], acc[:, 0:H])
                nc.gpsimd.dma_start(out_dram[jc * P : (jc + 1) * P, :], out_sb[:])

            pending = []  # (jc, atT) whose matmuls run two panels later
            ring = [nc.sync, nc.scalar]
            ring_bytes = [0, 0]
            for jc in range(NT):
                ntiles = NT - jc
                W = ntiles * P

                atT = at_pool.tile([P, W], F32R, tag="atT")
                for g0 in range(0, ntiles, GROUP):
                    gn = min(GROUP, ntiles - g0)
                    a_chunk = a_pool.tile([P, GROUP * P], DT, tag="apanel")
                    r = 0 if ring_bytes[0] <= ring_bytes[1] else 1
                    ring_bytes[r] += gn * P * P * 4
                    ring[r].dma_start(
                        a_chunk[:, 0 : gn * P],
                        a_dram[
                            jc * P : (jc + 1) * P,
                            (jc + g0) * P : (jc + g0 + gn) * P,
                        ],
                    )
                    tp = tp_pool.tile([P, GROUP * P], DT, tag="tp")
                    for k in range(gn):
                        nc.tensor.transpose(
                            tp[:, k * P : (k + 1) * P],
                            a_chunk[:, k * P : (k + 1) * P],
                            identity[:],
                        )
                    dst = atT[:, g0 * P : (g0 + gn) * P]
                    srcp = tp[:, 0 : gn * P]
                    if g0 == 0:
                        # Diagonal-containing chunk: mask during the
                        # f32r-rounding copy.
                        nc.vector.tensor_tensor(
                            dst, srcp, cmask[:, 0 : gn * P], mybir.AluOpType.mult
                        )
                    else:
                        nc.scalar.copy(dst, srcp)

                pending.append((jc, atT))
                if len(pending) > 2:
                    matmuls_and_store(*pending.pop(0))

            for item in pending:
                matmuls_and_store(*item)

    nc.finalize()
    return nc


def kernel(span_adjacency, bound_hidden):
    global LAST_RESULTS
    a = np.ascontiguousarray(np.asarray(span_adjacency, dtype=np.float32))
    h = np.ascontiguousarray(np.asarray(bound_hidden, dtype=np.float32))
    assert a.shape == (B, L, L) and h.shape == (B, L, H), (a.shape, h.shape)

    key = "full"
    if key not in _NC_CACHE:
        _NC_CACHE[key] = _build_nc()
    nc = _NC_CACHE[key]

    in_maps = [{"a": a[b], "h": h[b]} for b in range(B)]
    res = run_bass_kernel_spmd(
        nc,
        in_maps,
        core_ids=list(range(B)),
        trace=bool(os.environ.get("KERNEL_TRACE")),
    )
    LAST_RESULTS = res
    out = np.stack([res.results[b]["out"] for b in range(B)], axis=0)
    return out


# revision 11
# speedup vs baseline: 1.0350x; 1.0350x over previous
"""Trainium2 Bass kernel for nn_BoundarySeg (segment_reduce).

out[b, j, 0:H]   = sum_{i>=j} A[b, j, i] * h[b, i, :]
out[b, j, H:2H]  = h[b, j, :] * sum_{i>=j} A[b, j, i]

Shapes: A [8, 2048, 2048] f32, h [8, 2048, 256] f32 -> out [8, 2048, 512] f32.
Sharding: data-parallel over batch; core c computes batch c.

Per-core algorithm (L=2048 in 16 tiles of 128, H=256):
  - h loads on the gpsimd SWDGE queue (its own DMA path) with an
    in-flight fp32->f32r cast, with a ones column at [.., 256] so the
    masked row-sum falls out of the main matmul as an extra column
    (N=258 keeps f32r at 1 cycle/row).
  - A panels jc=0..15: only the upper panel A[jc, jc:] is loaded (two
    HWDGE rings, byte-balanced, GROUP=8 tiles per chunk), transposed on
    PE in fp32 through PSUM and rounded to f32r during the PSUM->SBUF
    copy (the BIR verifier requires f32r matmul inputs to come from a
    rounding producer); the diagonal-containing chunk is masked (keep
    i >= j) by that copy via tensor_tensor on DVE.
  - acc[j, n] += At_block^T @ h over i-tiles >= jc (f32r, N=258).
    first half = acc[:, 0:256] (DVE copy to SBUF); second half =
    h[j, :] * acc[:, 256] (ACT activation with SBUF rowsum scale);
    single 256 KB store per panel on the gpsimd SWDGE queue.
  - matmuls run two panels behind transposes so the in-order PE queue
    never stalls on PSUM->SBUF copies.
"""

import os
import sys

import numpy as np

sys.path.insert(0, "/opt/trn_rl_repo")

import concourse.bass as bass  # noqa: E402
import concourse.bacc as bacc  # noqa: E402
import concourse.tile as tile  # noqa: E402
from concourse import mybir  # noqa: E402
from concourse.bass_utils import run_bass_kernel_spmd  # noqa: E402
from concourse.masks import make_identity, make_lower_triangular  # noqa: E402

B, L, H = 8, 2048, 256
P = 128
GROUP = 8  # 128-col transposes batched per PSUM tile / copy

DT = mybir.dt.float32
F32R = mybir.dt.float32r

# Results of the last run (exec_time_ns etc.) for the test harness.
LAST_RESULTS = None
_NC_CACHE = {}


def _build_nc(L=L, H=H):
    NT = L // P
    HE = H + 2  # even N for f32r; col H = ones (rowsum), col H+1 unused

    nc = bacc.Bacc(None, target_bir_lowering=False)
    a_dram = nc.dram_tensor("a", [L, L], DT, kind="ExternalInput")
    h_dram = nc.dram_tensor("h", [L, H], DT, kind="ExternalInput")
    out_dram = nc.dram_tensor("out", [L, 2 * H], DT, kind="ExternalOutput")

    with tile.TileContext(nc) as tc:
        with (
            tc.tile_pool(name="const", bufs=1) as const_pool,
            tc.tile_pool(name="hpool", bufs=1) as h_pool,
            tc.tile_pool(name="apanel", bufs=5) as a_pool,
            tc.tile_pool(name="atT", bufs=3) as at_pool,
            tc.tile_pool(name="tp", bufs=3, space=bass.MemorySpace.PSUM) as tp_pool,
            tc.tile_pool(name="acc", bufs=2, space=bass.MemorySpace.PSUM) as acc_pool,
            tc.tile_pool(name="outsb", bufs=4) as out_pool,
            tc.tile_pool(name="small", bufs=3) as small_pool,
        ):
            # h on the gpsimd SWDGE queue (own DMA path) with in-flight
            # fp32->f32r cast.  Issued FIRST so the Pool queue reaches
            # them immediately (identity/mask generation comes after).
            # Ones columns (rowsum trick) are written before the DMAs so
            # any conservative write-ordering costs ~1us, not the whole
            # h load; memset cannot write f32r, hence the DVE copy.
            h_all = h_pool.tile([P, NT, HE], F32R)
            ones_stage = const_pool.tile([P, NT * 2], DT)
            nc.vector.memset(ones_stage[:], 1.0)
            nc.vector.tensor_copy(
                h_all[:, :, H:HE], ones_stage[:].rearrange("p (t c) -> p t c", c=2)
            )
            h_re = h_dram[:].rearrange("(t p) n -> p t n", p=P)
            for t0, t1 in ((0, 2), (2, 4), (4, 8), (8, 12), (12, 16)):
                nc.gpsimd.dma_start(
                    out=h_all[:, t0:t1, 0:H], in_=h_re[:, t0:t1, :]
                )

            identity = const_pool.tile([P, P], DT)
            make_identity(nc, identity[:])
            # Mask for the *transposed* diagonal block ([i(part), j(free)],
            # keep i >= j -> lower triangular); columns P.. multiply by 1.0.
            # Bounced through DVE so consumers depend on DVE, not Pool.
            mask_src = const_pool.tile([P, P], DT)
            make_lower_triangular(nc, mask_src[:], val=1.0, diag=True)
            cmask = const_pool.tile([P, GROUP * P], DT)
            nc.vector.tensor_copy(cmask[:, 0:P], mask_src[:])
            nc.vector.memset(cmask[:, P : GROUP * P], 1.0)

            # Warmup transpose: absorbs the Pool->PE wait for `identity`.
            wtp = tp_pool.tile([P, GROUP * P], DT, tag="tp")
            nc.tensor.transpose(wtp[:, 0:P], identity[:], identity[:])

            store_engines = [nc.gpsimd, nc.sync, nc.scalar]
            store_q = []  # (jc, out_sb) held 2 extra panels before issue

            def issue_store(jc, out_sb):
                # Round-robin stores across all three DMA paths; the
                # 2-panel lag ensures the HWDGE ring head never stalls
                # on an unfinished acc in front of A-chunk issues.
                store_engines[jc % 3].dma_start(
                    out_dram[jc * P : (jc + 1) * P, :], out_sb[:]
                )

            def matmuls_and_store(jc, atT):
                ntiles = NT - jc
                acc = acc_pool.tile([P, HE], DT, tag="acc")
                for k in range(ntiles):
                    nc.tensor.matmul(
                        acc[:],
                        atT[:, k * P : (k + 1) * P],
                        h_all[:, jc + k, :],
                        start=(k == 0),
                        stop=(k == ntiles - 1),
                    )
                out_sb = out_pool.tile([P, 2 * H], DT, tag="outsb")
                rowsum = small_pool.tile([P, 1], DT, tag="rowsum")
                nc.scalar.copy(rowsum[:], acc[:, H : H + 1])
                nc.scalar.activation(
                    out_sb[:, H : 2 * H],
                    h_all[:, jc, 0:H].bitcast(DT),
                    mybir.ActivationFunctionType.Identity,
                    scale=rowsum[:],
                )
                nc.vector.tensor_copy(out_sb[:, 0:H], acc[:, 0:H])
                store_q.append((jc, out_sb))
                if len(store_q) > 2:
                    issue_store(*store_q.pop(0))

            pending = []  # (jc, atT) whose matmuls run two panels later
            ring = [nc.sync, nc.scalar]
            ring_bytes = [0, 0]
            for jc in range(NT):
                ntiles = NT - jc
                W = ntiles * P

                atT = at_pool.tile([P, W], F32R, tag="atT")
                for g0 in range(0, ntiles, GROUP):
                    gn = min(GROUP, ntiles - g0)
                    a_chunk = a_pool.tile([P, GROUP * P], DT, tag="apanel")
                    r = 0 if ring_bytes[0] <= ring_bytes[1] else 1
                    ring_bytes[r] += gn * P * P * 4
                    ring[r].dma_start(
                        a_chunk[:, 0 : gn * P],
                        a_dram[
                            jc * P : (jc + 1) * P,
                            (jc + g0) * P : (jc + g0 + gn) * P,
                        ],
                    )
                    tp = tp_pool.tile([P, GROUP * P], DT, tag="tp")
                    for k in range(gn):
                        nc.tensor.transpose(
                            tp[:, k * P : (k + 1) * P],
                            a_chunk[:, k * P : (k + 1) * P],
                            identity[:],
                        )
                    dst = atT[:, g0 * P : (g0 + gn) * P]
                    srcp = tp[:, 0 : gn * P]
                    if g0 == 0:
                        # Diagonal-containing chunk: mask during the
                        # f32r-rounding copy.
                        nc.vector.tensor_tensor(
                            dst, srcp, cmask[:, 0 : gn * P], mybir.AluOpType.mult
                        )
                    else:
                        nc.scalar.copy(dst, srcp)

                pending.append((jc, atT))
                if len(pending) > 2:
                    matmuls_and_store(*pending.pop(0))

            for item in pending:
                matmuls_and_store(*item)
            for item in store_q:
                issue_store(*item)

    nc.finalize()
    return nc


def kernel(span_adjacency, bound_hidden):
    global LAST_RESULTS
    a = np.ascontiguousarray(np.asarray(span_adjacency, dtype=np.float32))
    h = np.ascontiguousarray(np.asarray(bound_hidden, dtype=np.float32))
    assert a.shape == (B, L, L) and h.shape == (B, L, H), (a.shape, h.shape)

    key = "full"
    if key not in _NC_CACHE:
        _NC_CACHE[key] = _build_nc()
    nc = _NC_CACHE[key]

    in_maps = [{"a": a[b], "h": h[b]} for b in range(B)]
    res = run_bass_kernel_spmd(
        nc,
        in_maps,
        core_ids=list(range(B)),
        trace=bool(os.environ.get("KERNEL_TRACE")),
    )
    LAST_RESULTS = res
    out = np.stack([res.results[b]["out"] for b in range(B)], axis=0)
    return out
